# revision 38
# baseline (speedup 1.0000x reference)
"""Trainium2 Bass kernel for a decoder block (MHA + GELU MLP, pre-LN, causal).

Problem shapes (hardcoded): B=2, T=2048, C=512, H=8, HD=64, f32 in/out.

Sharding: 8 cores = 2 batches x 4 interleaved query slots. Core (b, j)
handles query tiles at ORIGINAL 128-row positions {j, 4+j, 8+j, 12+j}.
Each core receives its batch's x rotated by 128*(3-j) rows so its query
tiles sit at rotated positions {3, 7, 11, 15}; causality becomes
core-invariant: key tile s is visible to query tile (4i+3) iff s <= 4i+3,
so the scores for key tile s cover the contiguous query-column suffix
[128*(s//4), 512). Key tiles at rotated position s < 3-j are "dead"
(wrapped future keys) and killed by a per-core exp bias of -1e30; the
diagonal (s % 4 == 3) gets a static triangular mask added via an
identity matmul. This computes 40/64 of the full score rectangle
uniformly on every core.

Attention/QKV matmul operands are bf16; the FFN runs in fp8e4 with
perf_mode=DoubleRow (weights scaled x16/x64 host-side, descaled via the
gelu scale and the final residual add); accumulation stays f32 in PSUM and
the residual stream stays f32. Scores for the two heads of a pair run
CONCURRENTLY on the two halves of the PE array (K=64 row-tiling via
base_partition 0/64), writing adjacent PSUM banks so one Exp activation
covers both. The softmax denominator rides along as a ones-column appended
to V; its row is bounced to SBUF on the Scalar engine, inverted with the
fast approximate reciprocal (exact DVE reciprocal is serial in free-size),
partition-broadcast on GpSimd, and each head-pair's tail is deferred two
iterations into the next pair's score loop. Transposes are plain matmuls
against a bf16 identity (streams 128 rows, keeps the PE HAM-warm) rather
than transpose-mode. Evictions alternate between Vector and Scalar engines
to balance the two; FFN weights are pre-staged in SBUF during attention
and FFN2 runs qt-major so each PSUM bank accumulates 8 back-to-back
DoubleRow matmuls.
"""

import os
import sys

for _p in ("/opt/trn_rl_repo",):
    if _p not in sys.path and os.path.isdir(_p):
        sys.path.insert(0, _p)

import ml_dtypes
import numpy as np

import concourse.bacc as bacc
import concourse.bass as bass
import concourse.tile as tile
from concourse import mybir
from concourse.bass_utils import run_bass_kernel_spmd

F32 = mybir.dt.float32
F32R = mybir.dt.float32r
BF16 = mybir.dt.bfloat16
FP8 = mybir.dt.float8e4
DR = mybir.MatmulPerfMode.DoubleRow
AF = mybir.ActivationFunctionType
MUL = mybir.AluOpType.mult

B, T, C, H, HD = 2, 2048, 512, 8, 64
NCORES = 8
QB = 512          # query rows per core (4 tiles of 128)
NT = T // 128     # 16 key tiles
NEG = -1.0e30
W1SCL = 16.0
W2SCL = 64.0

last_run = None       # test harness reads exec_time_ns from here
_prog_cache = {}


def r(ap):
    return ap.bitcast(mybir.dt.float32r)


def _build_program():
    nc = bacc.Bacc("TRN2", target_bir_lowering=False, debug=False,
                   num_devices=NCORES)

    xb_d = nc.dram_tensor("xb", [NT, 128, 512], BF16, kind="ExternalInput")
    xq_d = nc.dram_tensor("xq", [4, 128, 512], F32, kind="ExternalInput")
    wq_d = nc.dram_tensor("wq", [128, 4, 512], BF16, kind="ExternalInput")
    wk_d = nc.dram_tensor("wk", [128, 4, 512], BF16, kind="ExternalInput")
    wv_d = nc.dram_tensor("wv", [128, 4, 512], BF16, kind="ExternalInput")
    wo_d = nc.dram_tensor("wo", [128, 4, 512], BF16, kind="ExternalInput")
    w1_d = nc.dram_tensor("w1", [128, 4, 16, 128], FP8, kind="ExternalInput")
    w2_d = nc.dram_tensor("w2", [128, 16, 512], FP8, kind="ExternalInput")
    kb_d = nc.dram_tensor("kbias", [128, 16], F32, kind="ExternalInput")
    bo_d = nc.dram_tensor("bo", [1, 512], BF16, kind="ExternalInput")
    b1_d = nc.dram_tensor("b1c", [128, 16], F32, kind="ExternalInput")
    b2_d = nc.dram_tensor("b2r", [1, 512], FP8, kind="ExternalInput")
    id_d = nc.dram_tensor("identc", [128, 128], BF16, kind="ExternalInput")
    mk_d = nc.dram_tensor("maskc", [128, 128], BF16, kind="ExternalInput")
    on_d = nc.dram_tensor("onesc", [128, 512], BF16, kind="ExternalInput")
    out_d = nc.dram_tensor("out", [QB, C], F32, kind="ExternalOutput")

    with tile.TileContext(nc) as tc:
        with (
            tc.tile_pool(name="const", bufs=1) as const,
            tc.tile_pool(name="mid", bufs=1) as mid,
            tc.tile_pool(name="tp", bufs=7) as tp,
            tc.tile_pool(name="hp", bufs=7) as hp,
            tc.tile_pool(name="sp", bufs=4) as sp,
        ):
            # ---------------- early constants ----------------
            ident = const.tile([128, 128], BF16)
            wv_sb = const.tile([128, 4, 512], BF16)
            eps_sb = const.tile([128, 1], F32)
            nc.vector.memset(eps_sb[:], 1e-5)

            # ---------------- persistent mid tensors ----------------
            h1t_sb = mid.tile([128, 4, 2048], BF16)  # ln1(x)^T
            kt_sb = mid.tile([128, 4, 2048], BF16)   # K^T (head pair, 64+64)
            v_sb = mid.tile([128, 16, 520], BF16)    # V + ones col per head
            qt_sb = mid.tile([128, 4, 512], BF16)    # Q^T (q-concat order)
            xq_sb = mid.tile([128, 4, 512], F32)     # raw x of the q tiles
            ones_sb = const.tile([128, 512], BF16)
            nc.sync.dma_start(ones_sb[:], on_d[:])
            vones = (v_sb[:, :, :]
                     .rearrange("p a (h e) -> p a h e", e=65)[:, :, :, 64:65])
            nc.vector.tensor_copy(
                vones, ones_sb[:, 0:128]
                .rearrange("p (a h) -> p a h", h=8).unsqueeze(3))

            def layernorm_to(src_ap, dst_ap, apply_on_act=False):
                """LN stats on DVE, sqrt on ACT; apply on DVE or ACT."""
                st = sp.tile([128, 6], F32, tag="st")
                nc.vector.bn_stats(out=st[:], in_=src_ap)
                mv = sp.tile([128, 2], F32, tag="mv")
                nc.vector.bn_aggr(out=mv[:], in_=st[:])
                lg = sp.tile([128, 1], F32, tag="lg")
                nc.scalar.activation(out=lg[:], in_=mv[:, 1:2], func=AF.Sqrt,
                                     bias=eps_sb[:])
                rs = sp.tile([128, 1], F32, tag="rs")
                nc.vector.reciprocal(out=rs[:], in_=lg[:])
                if apply_on_act:
                    nm = sp.tile([128, 1], F32, tag="nm")
                    nc.vector.scalar_tensor_tensor(
                        out=nm[:], in0=mv[:, 0:1], scalar=-1.0, in1=rs[:],
                        op0=MUL, op1=MUL)
                    nc.scalar.activation(out=dst_ap, in_=src_ap,
                                         func=AF.Identity,
                                         bias=nm[:], scale=rs[:])
                else:
                    nc.vector.tensor_scalar(
                        out=dst_ap, in0=src_ap, scalar1=mv[:, 0:1],
                        scalar2=rs[:],
                        op0=mybir.AluOpType.subtract, op1=MUL)

            # ======== phase A: LN1 + transpose + Q/K/V ========
            with (
                tc.tile_pool(name="p1", bufs=1) as p1,
                tc.tile_pool(name="ptr", bufs=2, space="PSUM") as ptr,
                tc.tile_pool(name="ppv", bufs=2, space="PSUM") as ppv,
                tc.tile_pool(name="ppk", bufs=2, space="PSUM") as ppk,
                tc.tile_pool(name="ppq", bufs=1, space="PSUM") as ppq,
            ):
                wq_sb = p1.tile([128, 4, 512], BF16)
                wk_sb = p1.tile([128, 4, 512], BF16)

                for g in range(4):
                    # q-slot tile first so Q(g) is ready before the K chunk
                    tlist = [4 * g + 3, 4 * g, 4 * g + 1, 4 * g + 2]
                    hts = {}
                    for t in tlist:
                        xt = tp.tile([128, 512], BF16, tag="xt")
                        nc.sync.dma_start(xt[:], xb_d[t])
                        if g == 0 and t == tlist[0]:
                            nc.sync.dma_start(ident[:], id_d[:])
                            nc.sync.dma_start(wv_sb[:], wv_d[:])
                            nc.sync.dma_start(wq_sb[:], wq_d[:])
                            nc.sync.dma_start(wk_sb[:], wk_d[:])
                        if t % 4 == 3:
                            nc.sync.dma_start(xq_sb[:, g, :], xq_d[g])
                        ht = hp.tile([128, 512], BF16, tag="ht")
                        layernorm_to(xt[:], ht[:],
                                     apply_on_act=(t % 2 == 1))
                        hts[t] = ht
                    for ti, t in enumerate(tlist):
                        ht = hts[t]
                        pst = ptr.tile([128, 4, 128], F32, tag="tr")
                        for cc in range(4):
                            nc.tensor.matmul(
                                pst[:, cc, :], ht[:, bass.ts(cc, 128)],
                                ident[:], start=True, stop=True)
                        nc.scalar.copy(h1t_sb[:, :, bass.ts(t, 128)], pst[:])
                        psv = ppv.tile([128, 512], F32, tag="pv")
                        for cc in range(4):
                            nc.tensor.matmul(
                                psv[:], h1t_sb[:, cc, bass.ts(t, 128)],
                                wv_sb[:, cc, :],
                                start=(cc == 0), stop=(cc == 3))
                        ev = (v_sb[:, t, :]
                              .rearrange("p (h e) -> p h e", e=65)[:, :, 0:64])
                        sv = psv[:].rearrange("p (h e) -> p h e", e=64)
                        if t % 2 == 1:
                            nc.vector.tensor_copy(ev, sv)
                        else:
                            nc.scalar.copy(ev, sv)
                        if ti == 0:
                            # Q for query slot i=g (h1T tile 4g+3)
                            psq = ppq.tile([128, 4, 128], F32, tag="pq")
                            for pr in range(4):
                                for cc in range(4):
                                    nc.tensor.matmul(
                                        psq[:, pr, :],
                                        wq_sb[:, cc, bass.ts(pr, 128)],
                                        h1t_sb[:, cc,
                                               bass.ts(4 * g + 3, 128)],
                                        start=(cc == 0), stop=(cc == 3))
                            nc.scalar.copy(qt_sb[:, :, bass.ts(g, 128)],
                                           psq[:])

                    # K for key chunk g
                    for pr in range(4):
                        psk = ppk.tile([128, 512], F32, tag="pk")
                        for cc in range(4):
                            nc.tensor.matmul(
                                psk[:], wk_sb[:, cc, bass.ts(pr, 128)],
                                h1t_sb[:, cc, bass.ts(g, 512)],
                                start=(cc == 0), stop=(cc == 3))
                        ev = kt_sb[:, pr, bass.ts(g, 512)]
                        if pr < 2:
                            nc.vector.tensor_copy(ev, psk[:])
                        else:
                            nc.scalar.copy(ev, psk[:])
                    if g == 0:
                        kb_sb = const.tile([128, 16], F32)
                        nc.sync.dma_start(kb_sb[:], kb_d[:])
                        mask_sb = const.tile([128, 128], BF16)
                        nc.sync.dma_start(mask_sb[:], mk_d[:])
                    elif g == 1:
                        wo_sb = const.tile([128, 4, 512], BF16)
                        nc.sync.dma_start(wo_sb[:], wo_d[:])
                        bo_sb = const.tile([1, 512], BF16)
                        nc.sync.dma_start(bo_sb[:], bo_d[:])
                    elif g == 2:
                        b1_sb = const.tile([128, 16], F32)
                        nc.sync.dma_start(b1_sb[:], b1_d[:])
                        b2_sb = const.tile([1, 512], FP8)
                        nc.sync.dma_start(b2_sb[:], b2_d[:])

            # ======== phases B..C scope ========
            with tc.tile_pool(name="mid2", bufs=1) as mid2:
                at_sb = mid2.tile([128, 4, 512], BF16)   # attnT (normalized)
                x2_sb = mid2.tile([128, 4, 512], F32)    # post-attn residual
                h2t_sb = mid2.tile([128, 4, 512], FP8)   # ln2(x2)^T
                g_sb = mid2.tile([128, 16, 512], FP8)    # gelu(ffn1)^T
                # stage FFN weights during attention (SP queue is idle)
                w1_sb = mid2.tile([128, 4, 16, 128], FP8)
                nc.sync.dma_start(w1_sb[:], w1_d[:])
                w2_sb = mid2.tile([128, 16, 512], FP8)
                nc.sync.dma_start(w2_sb[:], w2_d[:])
                ones8 = mid2.tile([1, 128], FP8)
                nc.vector.memset(ones8[:], 1.0)

                # -------- phase B: attention --------
                with (
                    tc.tile_pool(name="psS", bufs=2, space="PSUM") as ps_ps,
                    tc.tile_pool(name="psO", bufs=2, space="PSUM") as po_ps,
                    tc.tile_pool(name="ap", bufs=6) as ap_pool,
                ):
                    po_tiles = {}

                    def attn_tail(pr):
                        po = po_tiles[pr]
                        for half in range(2):
                            dnr = ap_pool.tile([1, 512], F32, tag="dnr")
                            nc.scalar.copy(dnr[:],
                                           po[64:65, bass.ts(half, 512)])
                            rec = ap_pool.tile([1, 512], F32, tag="rec")
                            nc.vector.reciprocal_approx_fast(
                                out=rec[:], in_=dnr[:])
                            rb = ap_pool.tile([64, 512], F32, tag="rb")
                            nc.gpsimd.partition_broadcast(rb[:], rec[:])
                            nc.vector.tensor_mul(
                                out=at_sb[64 * half:64 * half + 64, pr, :],
                                in0=po[0:64, bass.ts(half, 512)],
                                in1=rb[:])

                    for pr in range(4):
                        po = po_ps.tile([65, 1024], F32, tag="po",
                                        name=f"po{pr}")
                        po_tiles[pr] = po
                        for s in range(NT):
                            q0 = 128 * (s // 4)
                            N = 512 - q0
                            diag = (s % 4 == 3)
                            psd = ps_ps.tile([128, 1024], F32, tag="ps")
                            for half in range(2):
                                b0 = 64 * half
                                o0 = (512 - N) if half == 0 else 512
                                nc.tensor.matmul(
                                    psd[:, o0:o0 + N],
                                    kt_sb[b0:b0 + 64, pr, bass.ts(s, 128)],
                                    qt_sb[b0:b0 + 64, pr, q0:512],
                                    start=True, stop=not diag,
                                    skip_group_check=True)
                            if diag:
                                nc.tensor.matmul(
                                    psd[:, 512 - N:512 - N + 128],
                                    ident[:], mask_sb[:],
                                    start=False, stop=True,
                                    skip_group_check=True)
                                nc.tensor.matmul(
                                    psd[:, 512:640],
                                    ident[:], mask_sb[:],
                                    start=False, stop=True,
                                    skip_group_check=True)
                            pt = ap_pool.tile([128, 1024], BF16, tag="pt")
                            nc.scalar.activation(
                                out=pt[:, 512 - N:512 + N],
                                in_=psd[:, 512 - N:512 + N], func=AF.Exp,
                                bias=kb_sb[:, s:s + 1])
                            for half in range(2):
                                h = 2 * pr + half
                                o0 = (512 - N) if half == 0 else 512
                                nc.tensor.matmul(
                                    po[:, 512 * half + q0:512 * half + 512],
                                    v_sb[:, s, h * 65:(h + 1) * 65],
                                    pt[:, o0:o0 + N],
                                    start=(s == 0), stop=(s == NT - 1),
                                    skip_group_check=True)
                            if s == 2 and pr > 0:
                                attn_tail(pr - 1)
                        if pr == 3:
                            attn_tail(3)

                with (
                    tc.tile_pool(name="pf", bufs=3, space="PSUM") as pf_ps,
                    tc.tile_pool(name="ptr2", bufs=2, space="PSUM") as ptr2,
                ):
                    # ---- proj + residual + LN2 + transpose, per qt ----
                    for qt in range(4):
                        ps = pf_ps.tile([128, 512], F32, tag="pf")
                        for cc in range(4):
                            nc.tensor.matmul(
                                ps[:], at_sb[:, cc, bass.ts(qt, 128)],
                                wo_sb[:, cc, :], start=(cc == 0), stop=False)
                        nc.tensor.matmul(ps[:], ones_sb[0:1, 0:128],
                                         bo_sb[:], start=False, stop=True)
                        nc.vector.tensor_add(out=x2_sb[:, qt, :], in0=ps[:],
                                             in1=xq_sb[:, qt, :])
                        ht = hp.tile([128, 512], BF16, tag="ht")
                        layernorm_to(x2_sb[:, qt, :], ht[:])
                        pst = ptr2.tile([128, 4, 128], F32, tag="tr")
                        for cc in range(4):
                            nc.tensor.matmul(
                                pst[:, cc, :], ht[:, bass.ts(cc, 128)],
                                ident[:], start=True, stop=True)
                        nc.scalar.copy(h2t_sb[:, :, bass.ts(qt, 128)],
                                       pst[:])

                    # -------- phase C3: FFN1 (fp8 DoubleRow) + gelu --------
                    for f in range(16):
                        ps = pf_ps.tile([128, 512], F32, tag="pf")
                        for k in range(0, 4, 2):
                            nc.tensor.matmul(
                                ps[:], w1_sb[:, k:k + 2, f, :],
                                h2t_sb[:, k:k + 2, :],
                                start=(k == 0), stop=(k == 2),
                                perf_mode=DR)
                        nc.scalar.activation(
                            out=g_sb[:, f, :], in_=ps[:], func=AF.Gelu,
                            bias=b1_sb[:, f:f + 1], scale=1.0 / W1SCL)

                    # -------- phase C4: FFN2 + residual + store --------
                    with (
                        tc.tile_pool(name="pf2", bufs=2,
                                     space="PSUM") as pf2_ps,
                        tc.tile_pool(name="op", bufs=2) as op,
                    ):
                        for qt in range(4):
                            pso = pf2_ps.tile([128, 512], F32, tag="o")
                            for ff in range(0, 16, 2):
                                nc.tensor.matmul(
                                    pso[:],
                                    g_sb[:, ff:ff + 2, bass.ts(qt, 128)],
                                    w2_sb[:, ff:ff + 2, :], start=(ff == 0),
                                    stop=False, skip_group_check=True,
                                    perf_mode=DR)
                            nc.tensor.matmul(
                                pso[:], ones8[:], b2_sb[:],
                                start=False, stop=True, skip_group_check=True)
                            ot = op.tile([128, 512], F32, tag="ot")
                            nc.vector.scalar_tensor_tensor(
                                out=ot[:], in0=pso[:], scalar=1.0 / W2SCL,
                                in1=x2_sb[:, qt, :],
                                op0=MUL, op1=mybir.AluOpType.add)
                            nc.sync.dma_start(out_d[bass.ts(qt, 128), :], ot[:])

    nc.compile()
    return nc


def _host_prep(x, Wq, Wk, Wv, Wo, bo, W1, b1, W2, b2, g1, be1, g2, be2):
    """Fold LN gains into weights; build per-core rotated inputs."""
    x = np.asarray(x, np.float32)
    g1 = np.asarray(g1, np.float32)
    be1 = np.asarray(be1, np.float32)
    g2 = np.asarray(g2, np.float32)
    be2 = np.asarray(be2, np.float32)

    wq_cat = np.transpose(np.asarray(Wq, np.float32), (1, 0, 2)).reshape(C, H * HD)
    wk_cat = np.transpose(np.asarray(Wk, np.float32), (1, 0, 2)).reshape(C, H * HD)
    wv_cat = np.transpose(np.asarray(Wv, np.float32), (1, 0, 2)).reshape(C, H * HD)
    scl = float(HD) ** -0.5
    wq_f = (g1[:, None] * wq_cat) * scl
    wk_f = g1[:, None] * wk_cat
    wv_f = g1[:, None] * wv_cat
    bq = (be1 @ wq_cat) * scl
    bk = be1 @ wk_cat
    bv = be1 @ wv_cat
    assert not (np.any(bq) or np.any(bk) or np.any(bv)), \
        "nonzero ln1 bias folding not supported"

    W1 = np.asarray(W1, np.float32)
    w1_f = g2[:, None] * W1
    b1_f = np.asarray(b1, np.float32) + be2 @ W1

    qidx = np.arange(128)[None, :]
    pidx = np.arange(128)[:, None]
    maskc = np.where(pidx <= qidx, 0.0, NEG).astype(np.float32)

    def bf(a):
        return np.ascontiguousarray(a).astype(ml_dtypes.bfloat16)

    def f8(a):
        return np.ascontiguousarray(a).astype(ml_dtypes.float8_e4m3fn)

    common = {
        "identc": bf(np.eye(128, dtype=np.float32)),
        "maskc": bf(maskc),
        "onesc": bf(np.ones((128, 512), np.float32)),
        "wq": bf(wq_f.reshape(4, 128, 512).transpose(1, 0, 2)),
        "wk": bf(wk_f.reshape(4, 128, 512).transpose(1, 0, 2)),
        "wv": bf(wv_f.reshape(4, 128, 512).transpose(1, 0, 2)),
        "wo": bf(np.asarray(Wo, np.float32)
                 .reshape(4, 128, 512).transpose(1, 0, 2)),
        "w1": f8(w1_f.reshape(4, 128, 16, 128).transpose(1, 0, 2, 3)
                 * W1SCL),
        "w2": f8(np.asarray(W2, np.float32).reshape(16, 128, 512)
                 .transpose(1, 0, 2) * W2SCL),
        "bo": bf(np.asarray(bo, np.float32).reshape(1, 512)),
        "b1c": np.ascontiguousarray(b1_f.reshape(16, 128).T),
        "b2r": f8(np.asarray(b2, np.float32).reshape(1, 512) * W2SCL),
    }

    in_maps = []
    for c in range(NCORES):
        bb, j = c // 4, c % 4
        xb_rot = np.roll(x[bb], 128 * (3 - j), axis=0)
        kbias = np.zeros(16, np.float32)
        kbias[:3 - j] = NEG
        xq = np.stack([x[bb][128 * (4 * i + j):128 * (4 * i + j) + 128]
                       for i in range(4)])
        im = dict(common)
        im["xb"] = bf(xb_rot.reshape(NT, 128, 512))
        im["xq"] = np.ascontiguousarray(xq)
        im["kbias"] = np.ascontiguousarray(
            np.broadcast_to(kbias.reshape(1, 16), (128, 16)))
        in_maps.append(im)
    return in_maps


def kernel(**inputs):
    global last_run
    in_maps = _host_prep(**inputs)
    if "prog" not in _prog_cache:
        _prog_cache["prog"] = _build_program()
    nc = _prog_cache["prog"]
    res = run_bass_kernel_spmd(nc, in_maps, list(range(NCORES)))
    last_run = res
    out = np.empty((B, T, C), np.float32)
    for c in range(NCORES):
        bb, j = c // 4, c % 4
        o = res.results[c]["out"]
        for i in range(4):
            out[bb, 128 * (4 * i + j):128 * (4 * i + j) + 128, :] = \
                o[128 * i:128 * (i + 1)]
    return out


# revision 39
# speedup vs baseline: 1.0375x; 1.0375x over previous
"""Trainium2 Bass kernel for a decoder block (MHA + GELU MLP, pre-LN, causal).

Problem shapes (hardcoded): B=2, T=2048, C=512, H=8, HD=64, f32 in/out.

Sharding: 8 cores = 2 batches x 4 interleaved query slots. Core (b, j)
handles query tiles at ORIGINAL 128-row positions {j, 4+j, 8+j, 12+j}.
Each core receives its batch's x rotated by 128*(3-j) rows so its query
tiles sit at rotated positions {3, 7, 11, 15}; causality becomes
core-invariant: key tile s is visible to query tile (4i+3) iff s <= 4i+3,
so the scores for key tile s cover the contiguous query-column suffix
[128*(s//4), 512). Key tiles at rotated position s < 3-j are "dead"
(wrapped future keys) and killed by a per-core exp bias of -1e30; the
diagonal (s % 4 == 3) gets a static triangular mask added via an
identity matmul. This computes 40/64 of the full score rectangle
uniformly on every core.

Attention/QKV matmul operands are bf16; the FFN runs in fp8e4 with
perf_mode=DoubleRow (weights scaled x16/x64 host-side, descaled via the
gelu scale and the final residual add); accumulation stays f32 in PSUM and
the residual stream stays f32. Scores for the two heads of a pair run
CONCURRENTLY on the two halves of the PE array (K=64 row-tiling via
base_partition 0/64), writing adjacent PSUM banks so one Exp activation
covers both. The softmax denominator rides along as a ones-column appended
to V; its row is bounced to SBUF on the Scalar engine, inverted with the
fast approximate reciprocal (exact DVE reciprocal is serial in free-size),
partition-broadcast on GpSimd, and each head-pair's tail is deferred two
iterations into the next pair's score loop. Transposes are plain matmuls
against a bf16 identity (streams 128 rows, keeps the PE HAM-warm) rather
than transpose-mode. Evictions alternate between Vector and Scalar engines
to balance the two; FFN weights are pre-staged in SBUF during attention
and FFN2 runs qt-major so each PSUM bank accumulates 8 back-to-back
DoubleRow matmuls.
"""

import os
import sys

for _p in ("/opt/trn_rl_repo",):
    if _p not in sys.path and os.path.isdir(_p):
        sys.path.insert(0, _p)

import ml_dtypes
import numpy as np

import concourse.bacc as bacc
import concourse.bass as bass
import concourse.tile as tile
from concourse import mybir
from concourse.bass_utils import run_bass_kernel_spmd

F32 = mybir.dt.float32
F32R = mybir.dt.float32r
BF16 = mybir.dt.bfloat16
FP8 = mybir.dt.float8e4
DR = mybir.MatmulPerfMode.DoubleRow
AF = mybir.ActivationFunctionType
MUL = mybir.AluOpType.mult

B, T, C, H, HD = 2, 2048, 512, 8, 64
NCORES = 8
QB = 512          # query rows per core (4 tiles of 128)
NT = T // 128     # 16 key tiles
NEG = -1.0e30
W1SCL = 16.0
W2SCL = 64.0

last_run = None       # test harness reads exec_time_ns from here
_prog_cache = {}


def r(ap):
    return ap.bitcast(mybir.dt.float32r)


def _build_program():
    nc = bacc.Bacc("TRN2", target_bir_lowering=False, debug=False,
                   num_devices=NCORES)

    xb_d = nc.dram_tensor("xb", [NT, 128, 512], BF16, kind="ExternalInput")
    xq_d = nc.dram_tensor("xq", [4, 128, 512], F32, kind="ExternalInput")
    wq_d = nc.dram_tensor("wq", [128, 4, 512], BF16, kind="ExternalInput")
    wk_d = nc.dram_tensor("wk", [128, 4, 512], BF16, kind="ExternalInput")
    wv_d = nc.dram_tensor("wv", [128, 4, 512], BF16, kind="ExternalInput")
    wo_d = nc.dram_tensor("wo", [128, 4, 512], BF16, kind="ExternalInput")
    w1_d = nc.dram_tensor("w1", [128, 4, 16, 128], FP8, kind="ExternalInput")
    w2_d = nc.dram_tensor("w2", [128, 16, 512], FP8, kind="ExternalInput")
    kb_d = nc.dram_tensor("kbias", [128, 16], F32, kind="ExternalInput")
    bo_d = nc.dram_tensor("bo", [1, 512], BF16, kind="ExternalInput")
    b1_d = nc.dram_tensor("b1c", [128, 16], F32, kind="ExternalInput")
    b2_d = nc.dram_tensor("b2r", [1, 512], FP8, kind="ExternalInput")
    id_d = nc.dram_tensor("identc", [128, 128], BF16, kind="ExternalInput")
    mk_d = nc.dram_tensor("maskc", [128, 128], BF16, kind="ExternalInput")
    on_d = nc.dram_tensor("onesc", [128, 512], BF16, kind="ExternalInput")
    out_d = nc.dram_tensor("out", [QB, C], F32, kind="ExternalOutput")

    with tile.TileContext(nc) as tc:
        with (
            tc.tile_pool(name="const", bufs=1) as const,
            tc.tile_pool(name="mid", bufs=1) as mid,
            tc.tile_pool(name="tp", bufs=5) as tp,
            tc.tile_pool(name="hp", bufs=5) as hp,
            tc.tile_pool(name="sp", bufs=4) as sp,
        ):
            # ---------------- early constants ----------------
            ident = const.tile([128, 128], BF16)
            wv_sb = const.tile([128, 4, 512], BF16)
            eps_sb = const.tile([128, 1], F32)
            nc.vector.memset(eps_sb[:], 1e-5)

            # ---------------- persistent mid tensors ----------------
            h1t_sb = mid.tile([128, 4, 2048], BF16)  # ln1(x)^T
            kt_sb = mid.tile([128, 4, 2048], BF16)   # K^T (head pair, 64+64)
            v_sb = mid.tile([128, 16, 520], BF16)    # V + ones col per head
            qt_sb = mid.tile([128, 4, 512], BF16)    # Q^T (q-concat order)
            xq_sb = mid.tile([128, 4, 512], F32)     # raw x of the q tiles
            ones_sb = const.tile([128, 512], BF16)
            nc.sync.dma_start(ones_sb[:], on_d[:])
            vones = (v_sb[:, :, :]
                     .rearrange("p a (h e) -> p a h e", e=65)[:, :, :, 64:65])
            nc.vector.tensor_copy(
                vones, ones_sb[:, 0:128]
                .rearrange("p (a h) -> p a h", h=8).unsqueeze(3))

            def layernorm_to(src_ap, dst_ap, apply_on_act=False):
                """LN stats on DVE, sqrt on ACT; apply on DVE or ACT."""
                st = sp.tile([128, 6], F32, tag="st")
                nc.vector.bn_stats(out=st[:], in_=src_ap)
                mv = sp.tile([128, 2], F32, tag="mv")
                nc.vector.bn_aggr(out=mv[:], in_=st[:])
                lg = sp.tile([128, 1], F32, tag="lg")
                nc.scalar.activation(out=lg[:], in_=mv[:, 1:2], func=AF.Sqrt,
                                     bias=eps_sb[:])
                rs = sp.tile([128, 1], F32, tag="rs")
                nc.vector.reciprocal(out=rs[:], in_=lg[:])
                if apply_on_act:
                    nm = sp.tile([128, 1], F32, tag="nm")
                    nc.vector.scalar_tensor_tensor(
                        out=nm[:], in0=mv[:, 0:1], scalar=-1.0, in1=rs[:],
                        op0=MUL, op1=MUL)
                    nc.scalar.activation(out=dst_ap, in_=src_ap,
                                         func=AF.Identity,
                                         bias=nm[:], scale=rs[:])
                else:
                    nc.vector.tensor_scalar(
                        out=dst_ap, in0=src_ap, scalar1=mv[:, 0:1],
                        scalar2=rs[:],
                        op0=mybir.AluOpType.subtract, op1=MUL)

            # ======== phase A: LN1 + transpose + Q/K/V ========
            with (
                tc.tile_pool(name="p1", bufs=1) as p1,
                tc.tile_pool(name="ptr", bufs=2, space="PSUM") as ptr,
                tc.tile_pool(name="ppv", bufs=2, space="PSUM") as ppv,
                tc.tile_pool(name="ppk", bufs=2, space="PSUM") as ppk,
                tc.tile_pool(name="ppq", bufs=1, space="PSUM") as ppq,
            ):
                wq_sb = p1.tile([128, 4, 512], BF16)
                wk_sb = p1.tile([128, 4, 512], BF16)

                for g in range(4):
                    # q-slot tile first so Q(g) is ready before the K chunk
                    tlist = [4 * g + 3, 4 * g, 4 * g + 1, 4 * g + 2]
                    hts = {}
                    for t in tlist:
                        xt = tp.tile([128, 512], BF16, tag="xt")
                        nc.sync.dma_start(xt[:], xb_d[t])
                        if g == 0 and t == tlist[0]:
                            nc.sync.dma_start(ident[:], id_d[:])
                            nc.sync.dma_start(wv_sb[:], wv_d[:])
                            nc.sync.dma_start(wq_sb[:], wq_d[:])
                            nc.sync.dma_start(wk_sb[:], wk_d[:])
                        if t % 4 == 3:
                            nc.sync.dma_start(xq_sb[:, g, :], xq_d[g])
                        ht = hp.tile([128, 512], BF16, tag="ht")
                        layernorm_to(xt[:], ht[:],
                                     apply_on_act=(t % 2 == 1))
                        hts[t] = ht
                    for ti, t in enumerate(tlist):
                        ht = hts[t]
                        pst = ptr.tile([128, 4, 128], F32, tag="tr")
                        for cc in range(4):
                            nc.tensor.matmul(
                                pst[:, cc, :], ht[:, bass.ts(cc, 128)],
                                ident[:], start=True, stop=True)
                        nc.scalar.copy(h1t_sb[:, :, bass.ts(t, 128)], pst[:])
                        psv = ppv.tile([128, 512], F32, tag="pv")
                        for cc in range(4):
                            nc.tensor.matmul(
                                psv[:], h1t_sb[:, cc, bass.ts(t, 128)],
                                wv_sb[:, cc, :],
                                start=(cc == 0), stop=(cc == 3))
                        ev = (v_sb[:, t, :]
                              .rearrange("p (h e) -> p h e", e=65)[:, :, 0:64])
                        sv = psv[:].rearrange("p (h e) -> p h e", e=64)
                        if t % 2 == 1:
                            nc.vector.tensor_copy(ev, sv)
                        else:
                            nc.scalar.copy(ev, sv)
                        if ti == 0:
                            # Q for query slot i=g (h1T tile 4g+3)
                            psq = ppq.tile([128, 4, 128], F32, tag="pq")
                            for pr in range(4):
                                for cc in range(4):
                                    nc.tensor.matmul(
                                        psq[:, pr, :],
                                        wq_sb[:, cc, bass.ts(pr, 128)],
                                        h1t_sb[:, cc,
                                               bass.ts(4 * g + 3, 128)],
                                        start=(cc == 0), stop=(cc == 3))
                            nc.scalar.copy(qt_sb[:, :, bass.ts(g, 128)],
                                           psq[:])

                    # K for key chunk g
                    for pr in range(4):
                        psk = ppk.tile([128, 512], F32, tag="pk")
                        for cc in range(4):
                            nc.tensor.matmul(
                                psk[:], wk_sb[:, cc, bass.ts(pr, 128)],
                                h1t_sb[:, cc, bass.ts(g, 512)],
                                start=(cc == 0), stop=(cc == 3))
                        ev = kt_sb[:, pr, bass.ts(g, 512)]
                        if pr < 2:
                            nc.vector.tensor_copy(ev, psk[:])
                        else:
                            nc.scalar.copy(ev, psk[:])
                    if g == 0:
                        kb_sb = const.tile([128, 16], F32)
                        nc.sync.dma_start(kb_sb[:], kb_d[:])
                        mask_sb = const.tile([128, 128], BF16)
                        nc.sync.dma_start(mask_sb[:], mk_d[:])
                    elif g == 1:
                        wo_sb = const.tile([128, 4, 512], BF16)
                        nc.sync.dma_start(wo_sb[:], wo_d[:])
                        bo_sb = const.tile([1, 512], BF16)
                        nc.sync.dma_start(bo_sb[:], bo_d[:])
                    elif g == 2:
                        b1_sb = const.tile([128, 16], F32)
                        nc.sync.dma_start(b1_sb[:], b1_d[:])
                        b2_sb = const.tile([1, 512], FP8)
                        nc.sync.dma_start(b2_sb[:], b2_d[:])

            # ======== phases B..C scope ========
            with tc.tile_pool(name="mid2", bufs=1) as mid2:
                at_sb = mid2.tile([128, 4, 512], BF16)   # attnT (normalized)
                x2_sb = mid2.tile([128, 4, 512], F32)    # post-attn residual
                h2t_sb = mid2.tile([128, 4, 512], FP8)   # ln2(x2)^T
                g_sb = mid2.tile([128, 16, 512], FP8)    # gelu(ffn1)^T
                # stage FFN weights during attention (SP queue is idle)
                w1_sb = mid2.tile([128, 4, 16, 128], FP8)
                nc.sync.dma_start(w1_sb[:], w1_d[:])
                w2_sb = mid2.tile([128, 16, 512], FP8)
                nc.sync.dma_start(w2_sb[:], w2_d[:])
                ones8 = mid2.tile([1, 128], FP8)
                nc.vector.memset(ones8[:], 1.0)

                # -------- phase B: attention --------
                with (
                    tc.tile_pool(name="psS", bufs=2, space="PSUM") as ps_ps,
                    tc.tile_pool(name="psO", bufs=2, space="PSUM") as po_ps,
                    tc.tile_pool(name="ap", bufs=6) as ap_pool,
                ):
                    po_tiles = {}

                    def attn_tail(pr):
                        po = po_tiles[pr]
                        dnr = ap_pool.tile([1, 1024], F32, tag="dnr")
                        nc.scalar.copy(dnr[:], po[64:65, :])
                        for half in range(2):
                            rec = ap_pool.tile([1, 512], F32, tag="rec")
                            nc.vector.reciprocal_approx_fast(
                                out=rec[:], in_=dnr[:, bass.ts(half, 512)])
                            rb = ap_pool.tile([64, 512], F32, tag="rb")
                            nc.gpsimd.partition_broadcast(rb[:], rec[:])
                            nc.vector.tensor_mul(
                                out=at_sb[64 * half:64 * half + 64, pr, :],
                                in0=po[0:64, bass.ts(half, 512)],
                                in1=rb[:])

                    for pr in range(4):
                        po = po_ps.tile([65, 1024], F32, tag="po",
                                        name=f"po{pr}")
                        po_tiles[pr] = po
                        for s in range(NT):
                            q0 = 128 * (s // 4)
                            N = 512 - q0
                            diag = (s % 4 == 3)
                            psd = ps_ps.tile([128, 1024], F32, tag="ps")
                            for half in range(2):
                                b0 = 64 * half
                                o0 = (512 - N) if half == 0 else 512
                                nc.tensor.matmul(
                                    psd[:, o0:o0 + N],
                                    kt_sb[b0:b0 + 64, pr, bass.ts(s, 128)],
                                    qt_sb[b0:b0 + 64, pr, q0:512],
                                    start=True, stop=not diag,
                                    skip_group_check=True)
                            if diag:
                                nc.tensor.matmul(
                                    psd[:, 512 - N:512 - N + 128],
                                    ident[:], mask_sb[:],
                                    start=False, stop=True,
                                    skip_group_check=True)
                                nc.tensor.matmul(
                                    psd[:, 512:640],
                                    ident[:], mask_sb[:],
                                    start=False, stop=True,
                                    skip_group_check=True)
                            pt = ap_pool.tile([128, 1024], BF16, tag="pt")
                            nc.scalar.activation(
                                out=pt[:, 512 - N:512 + N],
                                in_=psd[:, 512 - N:512 + N], func=AF.Exp,
                                bias=kb_sb[:, s:s + 1])
                            for half in range(2):
                                h = 2 * pr + half
                                o0 = (512 - N) if half == 0 else 512
                                nc.tensor.matmul(
                                    po[:, 512 * half + q0:512 * half + 512],
                                    v_sb[:, s, h * 65:(h + 1) * 65],
                                    pt[:, o0:o0 + N],
                                    start=(s == 0), stop=(s == NT - 1),
                                    skip_group_check=True)
                            if s == 2 and pr > 0:
                                attn_tail(pr - 1)
                        if pr == 3:
                            attn_tail(3)

                with (
                    tc.tile_pool(name="pf", bufs=3, space="PSUM") as pf_ps,
                    tc.tile_pool(name="ptr2", bufs=2, space="PSUM") as ptr2,
                ):
                    # ---- proj + residual + LN2 + transpose, per qt ----
                    for qt in range(4):
                        ps = pf_ps.tile([128, 512], F32, tag="pf")
                        for cc in range(4):
                            nc.tensor.matmul(
                                ps[:], at_sb[:, cc, bass.ts(qt, 128)],
                                wo_sb[:, cc, :], start=(cc == 0), stop=False)
                        nc.tensor.matmul(ps[:], ones_sb[0:1, 0:128],
                                         bo_sb[:], start=False, stop=True)
                        nc.vector.tensor_add(out=x2_sb[:, qt, :], in0=ps[:],
                                             in1=xq_sb[:, qt, :])
                        ht = hp.tile([128, 512], BF16, tag="ht")
                        layernorm_to(x2_sb[:, qt, :], ht[:])
                        pst = ptr2.tile([128, 4, 128], F32, tag="tr")
                        for cc in range(4):
                            nc.tensor.matmul(
                                pst[:, cc, :], ht[:, bass.ts(cc, 128)],
                                ident[:], start=True, stop=True)
                        nc.scalar.copy(h2t_sb[:, :, bass.ts(qt, 128)],
                                       pst[:])

                    # -------- phase C3: FFN1 (fp8 DoubleRow) + gelu --------
                    for f in range(16):
                        ps = pf_ps.tile([128, 512], F32, tag="pf")
                        for k in range(0, 4, 2):
                            nc.tensor.matmul(
                                ps[:], w1_sb[:, k:k + 2, f, :],
                                h2t_sb[:, k:k + 2, :],
                                start=(k == 0), stop=(k == 2),
                                perf_mode=DR)
                        nc.scalar.activation(
                            out=g_sb[:, f, :], in_=ps[:], func=AF.Gelu,
                            bias=b1_sb[:, f:f + 1], scale=1.0 / W1SCL)

                    # -------- phase C4: FFN2 + residual + store --------
                    with (
                        tc.tile_pool(name="pf2", bufs=2,
                                     space="PSUM") as pf2_ps,
                        tc.tile_pool(name="op", bufs=2) as op,
                    ):
                        for qt in range(4):
                            pso = pf2_ps.tile([128, 512], F32, tag="o")
                            for ff in range(0, 16, 2):
                                nc.tensor.matmul(
                                    pso[:],
                                    g_sb[:, ff:ff + 2, bass.ts(qt, 128)],
                                    w2_sb[:, ff:ff + 2, :], start=(ff == 0),
                                    stop=False, skip_group_check=True,
                                    perf_mode=DR)
                            nc.tensor.matmul(
                                pso[:], ones8[:], b2_sb[:],
                                start=False, stop=True, skip_group_check=True)
                            ot = op.tile([128, 512], F32, tag="ot")
                            nc.vector.scalar_tensor_tensor(
                                out=ot[:], in0=pso[:], scalar=1.0 / W2SCL,
                                in1=x2_sb[:, qt, :],
                                op0=MUL, op1=mybir.AluOpType.add)
                            nc.sync.dma_start(out_d[bass.ts(qt, 128), :], ot[:])

    nc.compile()
    return nc


def _host_prep(x, Wq, Wk, Wv, Wo, bo, W1, b1, W2, b2, g1, be1, g2, be2):
    """Fold LN gains into weights; build per-core rotated inputs."""
    x = np.asarray(x, np.float32)
    g1 = np.asarray(g1, np.float32)
    be1 = np.asarray(be1, np.float32)
    g2 = np.asarray(g2, np.float32)
    be2 = np.asarray(be2, np.float32)

    wq_cat = np.transpose(np.asarray(Wq, np.float32), (1, 0, 2)).reshape(C, H * HD)
    wk_cat = np.transpose(np.asarray(Wk, np.float32), (1, 0, 2)).reshape(C, H * HD)
    wv_cat = np.transpose(np.asarray(Wv, np.float32), (1, 0, 2)).reshape(C, H * HD)
    scl = float(HD) ** -0.5
    wq_f = (g1[:, None] * wq_cat) * scl
    wk_f = g1[:, None] * wk_cat
    wv_f = g1[:, None] * wv_cat
    bq = (be1 @ wq_cat) * scl
    bk = be1 @ wk_cat
    bv = be1 @ wv_cat
    assert not (np.any(bq) or np.any(bk) or np.any(bv)), \
        "nonzero ln1 bias folding not supported"

    W1 = np.asarray(W1, np.float32)
    w1_f = g2[:, None] * W1
    b1_f = np.asarray(b1, np.float32) + be2 @ W1

    qidx = np.arange(128)[None, :]
    pidx = np.arange(128)[:, None]
    maskc = np.where(pidx <= qidx, 0.0, NEG).astype(np.float32)

    def bf(a):
        return np.ascontiguousarray(a).astype(ml_dtypes.bfloat16)

    def f8(a):
        return np.ascontiguousarray(a).astype(ml_dtypes.float8_e4m3fn)

    common = {
        "identc": bf(np.eye(128, dtype=np.float32)),
        "maskc": bf(maskc),
        "onesc": bf(np.ones((128, 512), np.float32)),
        "wq": bf(wq_f.reshape(4, 128, 512).transpose(1, 0, 2)),
        "wk": bf(wk_f.reshape(4, 128, 512).transpose(1, 0, 2)),
        "wv": bf(wv_f.reshape(4, 128, 512).transpose(1, 0, 2)),
        "wo": bf(np.asarray(Wo, np.float32)
                 .reshape(4, 128, 512).transpose(1, 0, 2)),
        "w1": f8(w1_f.reshape(4, 128, 16, 128).transpose(1, 0, 2, 3)
                 * W1SCL),
        "w2": f8(np.asarray(W2, np.float32).reshape(16, 128, 512)
                 .transpose(1, 0, 2) * W2SCL),
        "bo": bf(np.asarray(bo, np.float32).reshape(1, 512)),
        "b1c": np.ascontiguousarray(b1_f.reshape(16, 128).T),
        "b2r": f8(np.asarray(b2, np.float32).reshape(1, 512) * W2SCL),
    }

    in_maps = []
    for c in range(NCORES):
        bb, j = c // 4, c % 4
        xb_rot = np.roll(x[bb], 128 * (3 - j), axis=0)
        kbias = np.zeros(16, np.float32)
        kbias[:3 - j] = NEG
        xq = np.stack([x[bb][128 * (4 * i + j):128 * (4 * i + j) + 128]
                       for i in range(4)])
        im = dict(common)
        im["xb"] = bf(xb_rot.reshape(NT, 128, 512))
        im["xq"] = np.ascontiguousarray(xq)
        im["kbias"] = np.ascontiguousarray(
            np.broadcast_to(kbias.reshape(1, 16), (128, 16)))
        in_maps.append(im)
    return in_maps


def kernel(**inputs):
    global last_run
    in_maps = _host_prep(**inputs)
    if "prog" not in _prog_cache:
        _prog_cache["prog"] = _build_program()
    nc = _prog_cache["prog"]
    res = run_bass_kernel_spmd(nc, in_maps, list(range(NCORES)))
    last_run = res
    out = np.empty((B, T, C), np.float32)
    for c in range(NCORES):
        bb, j = c // 4, c % 4
        o = res.results[c]["out"]
        for i in range(4):
            out[bb, 128 * (4 * i + j):128 * (4 * i + j) + 128, :] = \
                o[128 * i:128 * (i + 1)]
    return out
